# revision 59
# baseline (speedup 1.0000x reference)
"""MultiHeadAttention (B=2, S=2048, D=1024, 16 heads, causal, torch-.view head
split) on 8 TRN2 NeuronCores — v2 (restructured from the v1 baseline).

Sharding: core c handles batch b = c//4 and heads [4g, 4g+4) with g = c%4
(head h only touches token rows [128h, 128(h+1)) of its batch, so each core
needs just 512 rows of q/k/v). Wp is row-sharded by head; each core returns a
partial (2048, 1024) bf16 output and the host sums the 4 partials per batch.

v2 structural changes vs v1:
- PV matmuls are V-stationary (lhsT = V_nat [k,65], rhs = P^T [k, 512q]) so
  the moving operand is 512 wide and the output lands directly in att^T
  layout (d on partitions) — no PE transposes of the attention output, no
  per-s-loop DVE traffic. The 65th weight column of ones produces the softmax
  denominator as output row 64 for free.
- Softmax division: reciprocal of the denominator row (DVE), partition-
  broadcast (GpSimd), then one [64,512] multiply (DVE) writing attT2 bf16.
- Both heads of a pair share one [128,2,512] PSUM tile for QK^T (row-tiled
  concurrent matmuls) and ONE wide exp ACTIVATE covers both halves.
- PV matmuls are interleaved into the kt loop right after each exp so the
  PE never idles long enough for HAM to re-throttle.
- Projection PSUMs are paired ([128,2,512]) for wide psum->SBUF copies.
- Output partials are bf16 (host accumulates in f32).

Layout notes (same as v1):
- Head h's (2048, 64) matrices come from the (128 tokens x 1024 cols) block
  via s = 16*t + c, d = col%64, c = col//64. On-chip, head-space sequence
  order is PERMUTED within each 128-tile: w = 8*c + t_lo (t = 8*j+t_lo),
  making gather DMAs 32B-contiguous while preserving causal block structure.
  The final output DMA un-permutes.
"""

import numpy as np
import ml_dtypes
from contextlib import ExitStack

import concourse.bass as bass
import concourse.tile as tile
from concourse import bacc, mybir
from concourse.bass_utils import run_bass_kernel_spmd
from concourse.masks import make_identity

F32 = mybir.dt.float32
F16 = mybir.dt.float16
F16_NP = np.float16
BF16 = mybir.dt.bfloat16
BF16_NP = ml_dtypes.bfloat16

B, S, D, NH, HD = 2, 2048, 1024, 16, 64
HPC = 4          # heads per core
ROWS = 512       # token rows per core
N_CORES = 8
EXP_FN = mybir.ActivationFunctionType.Exp


def _perm_mask_np():
    """(128, 2, 128) bf16 mask in permuted within-tile coords, duplicated on
    axis 1 (one copy per head-half): mask[wk, :, wq] = 1 iff s(wq) >= s(wk),
    with s(w) = 16*(w%8) + w//8."""
    w = np.arange(128)
    s = 16 * (w % 8) + w // 8
    m = (s[None, :] >= s[:, None]).astype(np.float32)
    return np.ascontiguousarray(
        np.stack([m, m], axis=1)
    ).astype(BF16_NP)


_PROGRAM = None


def _build_program():
    nc = bacc.Bacc("TRN2", target_bir_lowering=False, debug=False)

    # host-side prepped layouts so every DMA line is contiguous:
    # x: [p, a, t] = X^T[128a+p, t]; W: [dblk, p, a, d] = W[128a+p, 128dblk+d]
    qT_d = nc.dram_tensor("qT", [128, 8, ROWS], F16, kind="ExternalInput").ap()
    kT_d = nc.dram_tensor("kT", [128, 8, ROWS], F16, kind="ExternalInput").ap()
    vT_d = nc.dram_tensor("vT", [128, 8, ROWS], BF16, kind="ExternalInput").ap()
    Wq_d = nc.dram_tensor("Wq", [8, 128, 8, 128], F16, kind="ExternalInput").ap()
    Wk_d = nc.dram_tensor("Wk", [8, 128, 8, 128], F16, kind="ExternalInput").ap()
    Wv_d = nc.dram_tensor("Wv", [8, 128, 8, 128], BF16, kind="ExternalInput").ap()
    Wp_d = nc.dram_tensor("Wp", [HPC * HD, D], BF16, kind="ExternalInput").ap()
    mask_d = nc.dram_tensor("mask", [128, 2, 128], BF16, kind="ExternalInput").ap()
    # permuted output [qt, ec, w, e]; the host un-permutes rows (w = 8c+tl
    # -> within-tile row 16tl+c) for free
    out_d = nc.dram_tensor(
        "out", [16, 2, 128, 512], BF16, kind="ExternalOutput"
    ).ap()

    with tile.TileContext(nc) as tc:
        with ExitStack() as ctx:
            # ---------------- persistent tensors ----------------
            pers = ctx.enter_context(tc.tile_pool(name="pers", bufs=1))
            phaseA = ctx.enter_context(tc.tile_pool(name="phaseA", bufs=1))
            # projected X^T, block layout: [p, dblk, t] = X^T[128*dblk+p, t]
            QT_sb = phaseA.tile([128, 8, ROWS], F16)
            KT_sb = phaseA.tile([128, 8, ROWS], F16)
            VT_sb = phaseA.tile([128, 8, ROWS], BF16)
            # head-gathered, pair-packed: [64*(h%2)+d, h//2, j, c, t_lo]
            QhT = pers.tile([128, 2, 16, 16, 8], F16)
            KhT = pers.tile([128, 2, 16, 16, 8], F16)
            V_pre = phaseA.tile([128, 2, 16, 16, 8], BF16)
            # partition-half-swapped copies of the projections
            QT_sw = phaseA.tile([128, 8, ROWS], F16)
            KT_sw = phaseA.tile([128, 8, ROWS], F16)
            VT_sw = phaseA.tile([128, 8, ROWS], BF16)
            # V natural per head + ones column: [w, hl, j, 0:65]
            V_nat = pers.tile([128, HPC, 16, HD + 1], BF16)
            # P^T = exp(S^T), per parity (= hpair): [k_w, half, kt, q]
            PT2 = [
                pers.tile([128, 2, 16, ROWS], BF16, name=f"PT2_{p}")
                for p in range(2)
            ]
            # att^T double-buffered by qc parity: [64*half+d, qcpar, pair, s, wq]
            attT2 = pers.tile([128, 2, 2, 4, 128], BF16)
            Wp_sb = pers.tile([128, 2, D], BF16)
            mask2 = pers.tile([128, 2, 128], BF16)
            ident = pers.tile([128, 128], BF16)
            make_identity(nc, ident)

            # (PV skips the never-written causal-masked columns of diagonal
            # slots, so no zero-memset of PT2 is needed)
            nc.gpsimd.memset(V_nat[:, :, :, HD : HD + 1], 1.0)

            # ---------------- PSUM pools ----------------
            # psS: paired [128,2,512] tiles (2 banks each) shared by the
            # projections and QK^T+exp. acc: PV accumulators. wp entered
            # after the phase-A-only vtr pool exits (banks are reused).
            psS = ctx.enter_context(
                tc.tile_pool(name="psS", bufs=2, space="PSUM")
            )
            accps = ctx.enter_context(
                tc.tile_pool(name="accps", bufs=2, space="PSUM")
            )

            sm_pool = ctx.enter_context(tc.tile_pool(name="small", bufs=4))
            bc_pool = ctx.enter_context(tc.tile_pool(name="bcast", bufs=2))
            out_pool = ctx.enter_context(tc.tile_pool(name="outt", bufs=4))

            def gather_batch(dst, src_sb, src_sw):
                """Head gather dst[64par+d, hp, j, c, tl] =
                src[64(c%2)+d, c//2, 128hl + 8j + tl] as partition-aligned
                DVE copies (multi-dim free APs), reading the half-swapped
                copy when par != c%2."""
                for hl in range(HPC):
                    par, hp = hl % 2, hl // 2
                    po = 64 * par
                    for c0 in range(2):
                        srct = src_sb if par == c0 else src_sw
                        inv = srct[
                            po : po + 64, :, 128 * hl : 128 * (hl + 1)
                        ].rearrange("d a (j w) -> d j a w", w=8)
                        outv = dst[po : po + 64, hp].rearrange(
                            "d j (cc c2) w -> d j cc c2 w", c2=2
                        )[:, :, :, c0, :]
                        nc.vector.tensor_copy(outv, inv)

            # ---------------- projections (q, k, v) ----------------
            with tc.tile_pool(name="xin", bufs=2) as xin_pool, tc.tile_pool(
                name="wcol", bufs=6
            ) as w_pool, tc.tile_pool(
                name="vtr", bufs=2, space="PSUM"
            ) as vtr_pool:
                proj = [
                    (qT_d, Wq_d, QT_sb, QT_sw, F16),
                    (kT_d, Wk_d, KT_sb, KT_sw, F16),
                    (vT_d, Wv_d, VT_sb, VT_sw, BF16),
                ]

                def st_exp(qc, hpair, kt, defer_masks=None):
                    """QK^T (row-tiled pair) + wide exp + diagonal mask."""
                    qoff = max(0, 128 * kt - 512 * qc)
                    ps2 = psS.tile([128, 2, ROWS], F32, tag="psS")
                    for half in range(2):
                        ho = 64 * half
                        nc.tensor.matmul(
                            ps2[:, half, qoff:ROWS],
                            lhsT=KhT[ho : ho + 64, hpair, kt, :, :],
                            rhs=QhT[
                                ho : ho + 64, hpair,
                                4 * qc + qoff // 128 : 4 * (qc + 1), :, :,
                            ],
                            start=True,
                            stop=True,
                        )
                    nc.scalar.activation(
                        PT2[hpair][:, :, kt, qoff:ROWS],
                        ps2[:, :, qoff:ROWS],
                        EXP_FN,
                    )
                    if kt >= 4 * qc:  # diagonal tile: causal mask
                        if defer_masks is not None:
                            defer_masks.append((hpair, kt, qoff))
                        else:
                            nc.gpsimd.tensor_mul(
                                PT2[hpair][:, :, kt, qoff : qoff + 128],
                                PT2[hpair][:, :, kt, qoff : qoff + 128],
                                mask2,
                            )

                for pi, (xd, wd, xt_out, xt_sw, xdt) in enumerate(proj):
                    x_in = xin_pool.tile([128, 8, ROWS], xdt, tag="x_in")
                    # x chunks + even weight columns dispatch on the sync
                    # ring, odd weight columns on the scalar ring: the first
                    # matmul group's inputs land ~2x sooner than with all
                    # 3MB serialized through one queue.
                    wcols = []
                    for dblk in range(8):
                        nc.sync.dma_start(
                            out=x_in[:, dblk, :], in_=xd[:, dblk, :]
                        )
                        wcol = w_pool.tile([128, 8, 128], xdt, tag="wcol")
                        # all weight columns on the scalar ring: the sync
                        # ring then carries only the 8 x chunks, so the dp0
                        # group's last contraction input lands ~2us sooner
                        nc.scalar.dma_start(out=wcol, in_=wd[dblk])
                        wcols.append(wcol)
                    if pi == 1:
                        # needed only for attention; dispatch after q/k DMAs
                        nc.scalar.dma_start(out=mask2, in_=mask_d)
                        nc.scalar.dma_start(
                            out=Wp_sb,
                            in_=Wp_d.rearrange("(a p) e -> p a e", p=128),
                        )
                    for dp in range(4):
                        ps2 = psS.tile([128, 2, ROWS], F32, tag="psS")
                        for h2 in range(2):
                            dblk = 2 * dp + h2
                            for mt in range(8):
                                nc.tensor.matmul(
                                    ps2[:, h2, :],
                                    lhsT=wcols[dblk][:, mt, :],
                                    rhs=x_in[:, mt, :],
                                    start=(mt == 0),
                                    stop=(mt == 7),
                                )
                        dst2 = xt_out[:, 2 * dp : 2 * dp + 2, :]
                        nc.scalar.copy(dst2, ps2)
                        # per-dp half-swap via the gpsimd ring (sync/scalar
                        # queues are busy streaming the next projection's
                        # x/w from DRAM)
                        sl = slice(2 * dp, 2 * dp + 2)
                        nc.gpsimd.dma_start(
                            out=xt_sw[0:64, sl], in_=xt_out[64:128, sl]
                        )
                        nc.gpsimd.dma_start(
                            out=xt_sw[64:128, sl], in_=xt_out[0:64, sl]
                        )
                    if xt_out is VT_sb:
                        # hoist qc=0's QK^T + exp first: they only wait on
                        # the q/k gathers, so exp starts while v's gather/
                        # transpose chain drains. Mask muls are deferred so
                        # they queue on DVE after v's gathers + V_nat copies.
                        deferred = []
                        for hpair in range(2):
                            for kt in range(4):
                                st_exp(0, hpair, kt, defer_masks=deferred)
                        gather_batch(V_pre, VT_sb, VT_sw)
                        # alternate partition halves so consecutive PE
                        # transposes land in different row groups and run
                        # concurrently (same mechanism as the QK pairs)
                        for hp in range(2):
                            for j in range(16):
                                for par in range(2):
                                    hl, ho = 2 * hp + par, 64 * par
                                    ps_v = vtr_pool.tile(
                                        [128, HD], BF16, tag="vtr"
                                    )
                                    nc.tensor.transpose(
                                        ps_v,
                                        V_pre[ho : ho + 64, hp, j, :, :],
                                        ident[ho : ho + 64, ho : ho + 64],
                                    )
                                    nc.vector.tensor_copy(
                                        V_nat[:, hl, j, 0:HD], ps_v
                                    )
                        for hpair, kt, qoff in deferred:
                            nc.gpsimd.tensor_mul(
                                PT2[hpair][:, :, kt, qoff : qoff + 128],
                                PT2[hpair][:, :, kt, qoff : qoff + 128],
                                mask2,
                            )
                    elif xt_out is KT_sb:
                        gather_batch(KhT, KT_sb, KT_sw)
                    else:
                        gather_batch(QhT, QT_sb, QT_sw)

            # ---------------- attention + output projection ----------------
            wpps = ctx.enter_context(
                tc.tile_pool(name="wpps", bufs=2, space="PSUM")
            )

            def emit_wp_s(qc, s):
                """Output projection for one q-tile of chunk qc. Filler
                emits (qc < 3) ship their outputs on the gpsimd ring so the
                sync ring carries only the boundary den/rrow DMAs (no
                head-of-line collisions); the tail's outs (qc == 3) are
                latency-critical and use the sync HWDGE ring."""
                qcpar = qc % 2
                qt = 4 * qc + s
                oring = nc.sync
                for ec in range(2):
                    ps_o = wpps.tile([128, ROWS], F32, tag="wp")
                    for pair in range(2):
                        nc.tensor.matmul(
                            ps_o,
                            lhsT=attT2[:, qcpar, pair, s, :],
                            rhs=Wp_sb[:, pair, 512 * ec : 512 * (ec + 1)],
                            start=(pair == 0),
                            stop=(pair == 1),
                        )
                    out_t = out_pool.tile([128, ROWS], BF16, tag="out_t")
                    nc.vector.tensor_copy(out_t, ps_o)
                    oring.dma_start(out=out_d[qt, ec], in_=out_t)

            def emit_wp(qc):
                for s in range(4):
                    emit_wp_s(qc, s)

            for qc in range(4):
                qcpar = qc % 2
                for hpair in range(2):
                    nkt = 4 * qc + 4
                    accs = []
                    for half in range(2):
                        acc_t = accps.tile([HD + 1, ROWS], F32, tag="acc")
                        accs.append(acc_t)
                    # lead the exp stream by 2 st_exps so the first PV
                    # pair finds PT2 ready sooner after the phase boundary
                    lead = min(2, 4 * qc)
                    for kt in range(lead):
                        st_exp(qc, hpair, kt)
                    for kt in range(nkt):
                        # this phase's diagonal slots (kt >= 4qc) were
                        # prefetched by the previous chunk (or phase A)
                        if kt + lead < 4 * qc:
                            st_exp(qc, hpair, kt + lead)
                        # PV accumulation, V-stationary: acc[d,q] += V^T P^T.
                        # Diagonal slots only stream their written columns;
                        # the masked-out prefix is exactly zero so skipping
                        # it is exact (and lets PT2 skip zero-memsets).
                        qo = max(0, 128 * kt - 512 * qc)
                        for half in range(2):
                            hl = 2 * hpair + half
                            nc.tensor.matmul(
                                accs[half][:, qo:ROWS],
                                lhsT=V_nat[:, hl, kt, :],
                                rhs=PT2[hpair][:, half, kt, qo:ROWS],
                                start=(kt == 0),
                                stop=(kt == nkt - 1),
                            )
                        # previous chunk's Wp as PE filler, one q-tile per
                        # slot. Slots start at kt=3: earlier fillers stall
                        # the PE queue head on attT2 pair-1 (the previous
                        # chunk's hpair-1 normalize lands ~3us into this
                        # phase). At qc=3 ALL four tiles are held back for
                        # the tail (qc3's loop is already ACT-saturated).
                        # Wp fillers run TWO chunks behind: each boundary's
                        # normalize chain takes ~9.5us to deliver attT2, so
                        # chunk qc-1's tiles would still stall mid-phase —
                        # chunk qc-2's are guaranteed ready. Wp(0,*) in
                        # qc2/hp0, Wp(1,*) in qc2/hp1; Wp(2,*)+Wp(3,*) in
                        # the tail (qc3's loop is already ACT-saturated).
                        if qc == 2 and kt >= 5 and (kt - 5) % 2 == 0:
                            sidx = (kt - 5) // 2
                            if sidx < 4:
                                emit_wp_s(hpair, sidx)
                    # Stage accs to SBUF immediately: the copies are the only
                    # thing the next phase's PV (acc-buffer WAR) waits on, so
                    # they go ahead of the prefetch block in the DVE queue.
                    last = qc == 3 and hpair == 1
                    if last:
                        # held-back Wp(2) tiles: PE work that hides the final
                        # normalize chain's DMA/reciprocal latency and keeps
                        # HAM warm through the tail. Emitted BEFORE the acc
                        # copies so their dependency watermark excludes the
                        # final boundary's DVE chain.
                        for s in range(4):
                            emit_wp_s(2, s)
                    acc_sbs, bcrecs = [], []
                    for half in range(2):
                        acc_sb = bc_pool.tile([HD + 1, ROWS], F32, tag="acc_sb")
                        nc.vector.tensor_copy(acc_sb, accs[half])
                        acc_sbs.append(acc_sb)
                    if qc < 3:
                        # prefetch the NEXT chunk's fresh diagonal slots of
                        # this parity: keeps the exp stream dense through the
                        # normalize/Wp phase boundary (slots are write-once
                        # here, first read by PV(qc+1, hpair))
                        for kt in range(4 * qc + 4, 4 * qc + 8):
                            st_exp(qc + 1, hpair, kt)
                    # normalize: attT2 = acc[0:64] * (1/denom) broadcast
                    for half in range(2):
                        acc_sb = acc_sbs[half]
                        # reciprocal of the denominator row: DVE recip cost
                        # scales with free-size per lane, so pack the 512
                        # values across 128 partitions first (tiny DMAs).
                        # The pack/unpack DMAs ride the sync HWDGE ring —
                        # near-idle in the attention phase and ~2us faster
                        # per transfer than the gpsimd SWDGE ring, which was
                        # stretching every boundary's attT2 chain to ~7.5us.
                        if last:
                            # final boundary: DMA-free den chain — PE
                            # transposes do the pack/unpack (the two small
                            # DMAs cost ~2us each in completion latency and
                            # left the PE idle long enough to re-throttle).
                            # PSUM scratch borrows the accps slots just
                            # freed by the acc_sb copies. bf16 rec path
                            # (~0.3% extra on the normalization, budget 2%).
                            denb = sm_pool.tile([1, ROWS], BF16, tag="rrow")
                            nc.vector.tensor_copy(denb, acc_sb[HD : HD + 1, :])
                            # [128, 4, 2] bf16: each transposed column at
                            # a 4-byte PSUM offset (alignment requirement)
                            ps_d = accps.tile(
                                [128, 4, 2], BF16, tag="acc", name="ps_d"
                            )
                            for s in range(4):
                                nc.tensor.transpose(
                                    ps_d[:, s, 0:1],
                                    denb[:, 128 * s : 128 * (s + 1)],
                                    ident[0:1, 0:1],
                                )
                            rec4 = sm_pool.tile([128, 4], F32, tag="rec128")
                            nc.vector.reciprocal(rec4, ps_d[:, :, 0])
                            rec4b = sm_pool.tile([128, 4], BF16, tag="den128")
                            nc.vector.tensor_copy(rec4b, rec4)
                            ps_r = accps.tile(
                                [128, ROWS], BF16, tag="acc", name="ps_r"
                            )
                            for s in range(4):
                                nc.tensor.transpose(
                                    ps_r[0:1, 128 * s : 128 * (s + 1)],
                                    rec4b[:, s : s + 1],
                                    ident,
                                )
                            rrowb = sm_pool.tile([1, ROWS], BF16, tag="rrow")
                            nc.vector.tensor_copy(rrowb, ps_r[0:1, :])
                            bcrec = bc_pool.tile([64, ROWS], BF16, tag="bcrec")
                            nc.gpsimd.partition_broadcast(bcrec, rrowb)
                            bcrecs.append(bcrec)
                            continue
                        den128 = sm_pool.tile([128, 4], F32, tag="den128")
                        nc.sync.dma_start(
                            out=den128,
                            in_=acc_sb[HD : HD + 1, :].rearrange(
                                "o (p e) -> o p e", p=128
                            ),
                        )
                        rec128 = sm_pool.tile([128, 4], F32, tag="rec128")
                        nc.vector.reciprocal(rec128, den128)
                        rrow = sm_pool.tile([1, ROWS], F32, tag="rrow")
                        nc.sync.dma_start(
                            out=rrow.rearrange("o (p e) -> o p e", p=128),
                            in_=rec128,
                        )
                        bcrec = bc_pool.tile([64, ROWS], F32, tag="bcrec")
                        nc.gpsimd.partition_broadcast(bcrec, rrow)
                        if not last:
                            dst = attT2[
                                64 * half : 64 * (half + 1), qcpar, hpair, :, :
                            ].rearrange("d s w -> d (s w)")
                            nc.vector.tensor_mul(dst, acc_sb[0:HD, :], bcrec)
                        else:
                            bcrecs.append(bcrec)
                    if last:
                        # final phase: normalize per s-tile so each Wp q-tile
                        # can start as soon as its slice is ready (shrinks
                        # the serial tail after the last exp)
                        for s in range(4):
                            for half in range(2):
                                dst = attT2[
                                    64 * half : 64 * (half + 1),
                                    qcpar, hpair, s, :,
                                ]
                                nc.vector.tensor_mul(
                                    dst,
                                    acc_sbs[half][0:HD, 128 * s : 128 * (s + 1)],
                                    bcrecs[half][:, 128 * s : 128 * (s + 1)],
                                )
                            emit_wp_s(3, s)

    nc.compile()
    return nc


def get_program():
    global _PROGRAM
    if _PROGRAM is None:
        _PROGRAM = _build_program()
    return _PROGRAM


def _prep_x(x, dt):
    """[512 rows, 1024] -> x^T block layout [p, a, t], contiguous."""
    xt = np.ascontiguousarray(np.asarray(x, np.float32).T)  # [1024, 512]
    return np.ascontiguousarray(
        xt.reshape(8, 128, ROWS).transpose(1, 0, 2)
    ).astype(dt)


def _prep_w(W, dt):
    """[1024, 1024] -> [dblk, p, a, d] with W[128a+p, 128dblk+d]."""
    Wf = np.asarray(W, np.float32).reshape(8, 128, 8, 128)
    return np.ascontiguousarray(Wf.transpose(2, 1, 0, 3)).astype(dt)


def make_in_maps(q, k, v, Wq, Wk, Wv, Wp):
    mask = _perm_mask_np()
    Wq_b = _prep_w(Wq, F16_NP)
    Wk_b = _prep_w(Wk, F16_NP)
    Wv_b = _prep_w(Wv, BF16_NP)
    Wp_f = np.asarray(Wp, np.float32)
    in_maps = []
    for core in range(N_CORES):
        b, g = divmod(core, 4)
        rows = slice(ROWS * g, ROWS * (g + 1))
        in_maps.append(
            {
                "qT": _prep_x(np.asarray(q[b])[rows], F16_NP),
                "kT": _prep_x(np.asarray(k[b])[rows], F16_NP),
                "vT": _prep_x(np.asarray(v[b])[rows], BF16_NP),
                "Wq": Wq_b,
                "Wk": Wk_b,
                "Wv": Wv_b,
                "Wp": np.ascontiguousarray(
                    Wp_f[HPC * HD * g : HPC * HD * (g + 1)]
                ).astype(BF16_NP),
                "mask": mask,
            }
        )
    return in_maps


def kernel(q, k, v, Wq, Wk, Wv, Wp, _trace=False, _trace_kwargs=None):
    nc = get_program()
    in_maps = make_in_maps(q, k, v, Wq, Wk, Wv, Wp)
    res = run_bass_kernel_spmd(
        nc,
        in_maps,
        core_ids=list(range(N_CORES)),
        trace=_trace,
        **(_trace_kwargs or {}),
    )
    # un-permute device layout [qt, ec, w=8c+tl, e] -> [s=128qt+16tl+c, 1024]
    s_in_tile = np.arange(128)
    w_of_s = 8 * (s_in_tile % 16) + s_in_tile // 16
    outs = []
    for c in range(N_CORES):
        o = res.results[c]["out"].astype(np.float32)  # [16, 2, 128, 512]
        o = o[:, :, w_of_s, :].transpose(0, 2, 1, 3).reshape(S, D)
        outs.append(o)
    full = np.stack(
        [
            outs[0] + outs[1] + outs[2] + outs[3],
            outs[4] + outs[5] + outs[6] + outs[7],
        ]
    ).astype(np.float32)
    if _trace:
        kernel._last_result = res
    return full



# revision 61
# speedup vs baseline: 1.3907x; 1.3907x over previous
"""MultiHeadAttention (B=2, S=2048, D=1024, 16 heads, causal, torch-.view head
split) on 8 TRN2 NeuronCores — v2 (restructured from the v1 baseline).

Sharding: core c handles batch b = c//4 and heads [4g, 4g+4) with g = c%4
(head h only touches token rows [128h, 128(h+1)) of its batch, so each core
needs just 512 rows of q/k/v). Wp is row-sharded by head; each core returns a
partial (2048, 1024) bf16 output and the host sums the 4 partials per batch.

v2 structural changes vs v1:
- PV matmuls are V-stationary (lhsT = V_nat [k,65], rhs = P^T [k, 512q]) so
  the moving operand is 512 wide and the output lands directly in att^T
  layout (d on partitions) — no PE transposes of the attention output, no
  per-s-loop DVE traffic. The 65th weight column of ones produces the softmax
  denominator as output row 64 for free.
- Softmax division: reciprocal of the denominator row (DVE), partition-
  broadcast (GpSimd), then one [64,512] multiply (DVE) writing attT2 bf16.
- Both heads of a pair share one [128,2,512] PSUM tile for QK^T (row-tiled
  concurrent matmuls) and ONE wide exp ACTIVATE covers both halves.
- PV matmuls are interleaved into the kt loop right after each exp so the
  PE never idles long enough for HAM to re-throttle.
- Projection PSUMs are paired ([128,2,512]) for wide psum->SBUF copies.
- Output partials are bf16 (host accumulates in f32).

Layout notes (same as v1):
- Head h's (2048, 64) matrices come from the (128 tokens x 1024 cols) block
  via s = 16*t + c, d = col%64, c = col//64. On-chip, head-space sequence
  order is PERMUTED within each 128-tile: w = 8*c + t_lo (t = 8*j+t_lo),
  making gather DMAs 32B-contiguous while preserving causal block structure.
  The final output DMA un-permutes.
"""

import numpy as np
import ml_dtypes
from contextlib import ExitStack

import concourse.bass as bass
import concourse.tile as tile
from concourse import bacc, mybir
from concourse.bass_utils import run_bass_kernel_spmd
from concourse.masks import make_identity

F32 = mybir.dt.float32
F16 = mybir.dt.float16
F16_NP = np.float16
BF16 = mybir.dt.bfloat16
BF16_NP = ml_dtypes.bfloat16

B, S, D, NH, HD = 2, 2048, 1024, 16, 64
HPC = 4          # heads per core
ROWS = 512       # token rows per core
N_CORES = 8
EXP_FN = mybir.ActivationFunctionType.Exp


def _perm_mask_np():
    """(128, 2, 128) bf16 mask in permuted within-tile coords, duplicated on
    axis 1 (one copy per head-half): mask[wk, :, wq] = 1 iff s(wq) >= s(wk),
    with s(w) = 16*(w%8) + w//8."""
    w = np.arange(128)
    s = 16 * (w % 8) + w // 8
    m = (s[None, :] >= s[:, None]).astype(np.float32)
    return np.ascontiguousarray(
        np.stack([m, m], axis=1)
    ).astype(BF16_NP)


_PROGRAM = None


def _build_program():
    nc = bacc.Bacc("TRN2", target_bir_lowering=False, debug=False)

    # host-side prepped layouts so every DMA line is contiguous:
    # x: [p, a, t] = X^T[128a+p, t]; W: [dblk, p, a, d] = W[128a+p, 128dblk+d]
    qT_d = nc.dram_tensor("qT", [128, 8, ROWS], F16, kind="ExternalInput").ap()
    kT_d = nc.dram_tensor("kT", [128, 8, ROWS], F16, kind="ExternalInput").ap()
    vT_d = nc.dram_tensor("vT", [128, 8, ROWS], BF16, kind="ExternalInput").ap()
    Wq_d = nc.dram_tensor("Wq", [8, 128, 8, 128], F16, kind="ExternalInput").ap()
    Wk_d = nc.dram_tensor("Wk", [8, 128, 8, 128], F16, kind="ExternalInput").ap()
    Wv_d = nc.dram_tensor("Wv", [8, 128, 8, 128], BF16, kind="ExternalInput").ap()
    Wp_d = nc.dram_tensor("Wp", [HPC * HD, D], BF16, kind="ExternalInput").ap()
    mask_d = nc.dram_tensor("mask", [128, 2, 128], BF16, kind="ExternalInput").ap()
    # permuted output [qt, ec, w, e]; the host un-permutes rows (w = 8c+tl
    # -> within-tile row 16tl+c) for free
    out_d = nc.dram_tensor(
        "out", [16, 2, 128, 512], BF16, kind="ExternalOutput"
    ).ap()

    with tile.TileContext(nc) as tc:
        with ExitStack() as ctx:
            # ---------------- persistent tensors ----------------
            pers = ctx.enter_context(tc.tile_pool(name="pers", bufs=1))
            phaseA = ctx.enter_context(tc.tile_pool(name="phaseA", bufs=1))
            # projected X^T, block layout: [p, dblk, t] = X^T[128*dblk+p, t]
            QT_sb = phaseA.tile([128, 8, ROWS], F16)
            KT_sb = phaseA.tile([128, 8, ROWS], F16)
            VT_sb = phaseA.tile([128, 8, ROWS], BF16)
            # head-gathered, pair-packed: [64*(h%2)+d, h//2, j, c, t_lo]
            QhT = pers.tile([128, 2, 16, 16, 8], F16)
            KhT = pers.tile([128, 2, 16, 16, 8], F16)
            V_pre = phaseA.tile([128, 2, 16, 16, 8], BF16)
            # partition-half-swapped copies of the projections
            QT_sw = phaseA.tile([128, 8, ROWS], F16)
            KT_sw = phaseA.tile([128, 8, ROWS], F16)
            VT_sw = phaseA.tile([128, 8, ROWS], BF16)
            # V natural per head + ones column: [w, hl, j, 0:65]
            V_nat = pers.tile([128, HPC, 16, HD + 1], BF16)
            # P^T = exp(S^T), per parity (= hpair): [k_w, half, kt, q]
            PT2 = [
                pers.tile([128, 2, 16, ROWS], BF16, name=f"PT2_{p}")
                for p in range(2)
            ]
            # att^T double-buffered by qc parity: [64*half+d, qcpar, pair, s, wq]
            attT2 = pers.tile([128, 2, 2, 4, 128], BF16)
            Wp_sb = pers.tile([128, 2, D], BF16)
            mask2 = pers.tile([128, 2, 128], BF16)
            ident = pers.tile([128, 128], BF16)
            make_identity(nc, ident)

            # (PV skips the never-written causal-masked columns of diagonal
            # slots, so no zero-memset of PT2 is needed)
            nc.gpsimd.memset(V_nat[:, :, :, HD : HD + 1], 1.0)

            # ---------------- PSUM pools ----------------
            # psS: paired [128,2,512] tiles (2 banks each) shared by the
            # projections and QK^T+exp. acc: PV accumulators. wp entered
            # after the phase-A-only vtr pool exits (banks are reused).
            psS = ctx.enter_context(
                tc.tile_pool(name="psS", bufs=2, space="PSUM")
            )
            accps = ctx.enter_context(
                tc.tile_pool(name="accps", bufs=2, space="PSUM")
            )

            sm_pool = ctx.enter_context(tc.tile_pool(name="small", bufs=4))
            bc_pool = ctx.enter_context(tc.tile_pool(name="bcast", bufs=2))
            out_pool = ctx.enter_context(tc.tile_pool(name="outt", bufs=4))

            def gather_batch(dst, src_sb, src_sw):
                """Head gather dst[64par+d, hp, j, c, tl] =
                src[64(c%2)+d, c//2, 128hl + 8j + tl] as partition-aligned
                DVE copies (multi-dim free APs), reading the half-swapped
                copy when par != c%2."""
                for hl in range(HPC):
                    par, hp = hl % 2, hl // 2
                    po = 64 * par
                    for c0 in range(2):
                        srct = src_sb if par == c0 else src_sw
                        inv = srct[
                            po : po + 64, :, 128 * hl : 128 * (hl + 1)
                        ].rearrange("d a (j w) -> d j a w", w=8)
                        outv = dst[po : po + 64, hp].rearrange(
                            "d j (cc c2) w -> d j cc c2 w", c2=2
                        )[:, :, :, c0, :]
                        nc.vector.tensor_copy(outv, inv)

            # ---------------- projections (q, k, v) ----------------
            with tc.tile_pool(name="xin", bufs=2) as xin_pool, tc.tile_pool(
                name="wcol", bufs=6
            ) as w_pool, tc.tile_pool(
                name="vtr", bufs=2, space="PSUM"
            ) as vtr_pool:
                proj = [
                    (qT_d, Wq_d, QT_sb, QT_sw, F16),
                    (kT_d, Wk_d, KT_sb, KT_sw, F16),
                    (vT_d, Wv_d, VT_sb, VT_sw, BF16),
                ]

                def st_exp(qc, hpair, kt, defer_masks=None):
                    """QK^T (row-tiled pair) + wide exp + diagonal mask."""
                    qoff = max(0, 128 * kt - 512 * qc)
                    ps2 = psS.tile([128, 2, ROWS], F32, tag="psS")
                    for half in range(2):
                        ho = 64 * half
                        nc.tensor.matmul(
                            ps2[:, half, qoff:ROWS],
                            lhsT=KhT[ho : ho + 64, hpair, kt, :, :],
                            rhs=QhT[
                                ho : ho + 64, hpair,
                                4 * qc + qoff // 128 : 4 * (qc + 1), :, :,
                            ],
                            start=True,
                            stop=True,
                        )
                    nc.scalar.activation(
                        PT2[hpair][:, :, kt, qoff:ROWS],
                        ps2[:, :, qoff:ROWS],
                        EXP_FN,
                    )
                    if kt >= 4 * qc:  # diagonal tile: causal mask
                        if defer_masks is not None:
                            defer_masks.append((hpair, kt, qoff))
                        else:
                            nc.vector.tensor_mul(
                                PT2[hpair][:, :, kt, qoff : qoff + 128],
                                PT2[hpair][:, :, kt, qoff : qoff + 128],
                                mask2,
                            )

                for pi, (xd, wd, xt_out, xt_sw, xdt) in enumerate(proj):
                    x_in = xin_pool.tile([128, 8, ROWS], xdt, tag="x_in")
                    # x chunks + even weight columns dispatch on the sync
                    # ring, odd weight columns on the scalar ring: the first
                    # matmul group's inputs land ~2x sooner than with all
                    # 3MB serialized through one queue.
                    wcols = []
                    for dblk in range(8):
                        nc.sync.dma_start(
                            out=x_in[:, dblk, :], in_=xd[:, dblk, :]
                        )
                        wcol = w_pool.tile([128, 8, 128], xdt, tag="wcol")
                        # all weight columns on the scalar ring: the sync
                        # ring then carries only the 8 x chunks, so the dp0
                        # group's last contraction input lands ~2us sooner
                        nc.scalar.dma_start(out=wcol, in_=wd[dblk])
                        wcols.append(wcol)
                    if pi == 1:
                        # needed only for attention; dispatch after q/k DMAs
                        nc.scalar.dma_start(out=mask2, in_=mask_d)
                        nc.scalar.dma_start(
                            out=Wp_sb,
                            in_=Wp_d.rearrange("(a p) e -> p a e", p=128),
                        )
                    for dp in range(4):
                        ps2 = psS.tile([128, 2, ROWS], F32, tag="psS")
                        for h2 in range(2):
                            dblk = 2 * dp + h2
                            for mt in range(8):
                                nc.tensor.matmul(
                                    ps2[:, h2, :],
                                    lhsT=wcols[dblk][:, mt, :],
                                    rhs=x_in[:, mt, :],
                                    start=(mt == 0),
                                    stop=(mt == 7),
                                )
                        dst2 = xt_out[:, 2 * dp : 2 * dp + 2, :]
                        nc.scalar.copy(dst2, ps2)
                        # per-dp half-swap via the gpsimd ring (sync/scalar
                        # queues are busy streaming the next projection's
                        # x/w from DRAM)
                        sl = slice(2 * dp, 2 * dp + 2)
                        nc.gpsimd.dma_start(
                            out=xt_sw[0:64, sl], in_=xt_out[64:128, sl]
                        )
                        nc.gpsimd.dma_start(
                            out=xt_sw[64:128, sl], in_=xt_out[0:64, sl]
                        )
                    if xt_out is VT_sb:
                        # hoist qc=0's QK^T + exp first: they only wait on
                        # the q/k gathers, so exp starts while v's gather/
                        # transpose chain drains. Mask muls are deferred so
                        # they queue on DVE after v's gathers + V_nat copies.
                        deferred = []
                        for hpair in range(2):
                            for kt in range(4):
                                st_exp(0, hpair, kt, defer_masks=deferred)
                        gather_batch(V_pre, VT_sb, VT_sw)
                        # alternate partition halves so consecutive PE
                        # transposes land in different row groups and run
                        # concurrently (same mechanism as the QK pairs)
                        for hp in range(2):
                            for j in range(16):
                                for par in range(2):
                                    hl, ho = 2 * hp + par, 64 * par
                                    ps_v = vtr_pool.tile(
                                        [128, HD], BF16, tag="vtr"
                                    )
                                    nc.tensor.transpose(
                                        ps_v,
                                        V_pre[ho : ho + 64, hp, j, :, :],
                                        ident[ho : ho + 64, ho : ho + 64],
                                    )
                                    nc.vector.tensor_copy(
                                        V_nat[:, hl, j, 0:HD], ps_v
                                    )
                        for hpair, kt, qoff in deferred:
                            nc.vector.tensor_mul(
                                PT2[hpair][:, :, kt, qoff : qoff + 128],
                                PT2[hpair][:, :, kt, qoff : qoff + 128],
                                mask2,
                            )
                    elif xt_out is KT_sb:
                        gather_batch(KhT, KT_sb, KT_sw)
                    else:
                        gather_batch(QhT, QT_sb, QT_sw)

            # ---------------- attention + output projection ----------------
            wpps = ctx.enter_context(
                tc.tile_pool(name="wpps", bufs=2, space="PSUM")
            )

            def emit_wp_s(qc, s):
                """Output projection for one q-tile of chunk qc. Filler
                emits (qc < 3) ship their outputs on the gpsimd ring so the
                sync ring carries only the boundary den/rrow DMAs (no
                head-of-line collisions); the tail's outs (qc == 3) are
                latency-critical and use the sync HWDGE ring."""
                qcpar = qc % 2
                qt = 4 * qc + s
                oring = nc.sync
                for ec in range(2):
                    ps_o = wpps.tile([128, ROWS], F32, tag="wp")
                    for pair in range(2):
                        nc.tensor.matmul(
                            ps_o,
                            lhsT=attT2[:, qcpar, pair, s, :],
                            rhs=Wp_sb[:, pair, 512 * ec : 512 * (ec + 1)],
                            start=(pair == 0),
                            stop=(pair == 1),
                        )
                    out_t = out_pool.tile([128, ROWS], BF16, tag="out_t")
                    nc.vector.tensor_copy(out_t, ps_o)
                    oring.dma_start(out=out_d[qt, ec], in_=out_t)

            def emit_wp(qc):
                for s in range(4):
                    emit_wp_s(qc, s)

            for qc in range(4):
                qcpar = qc % 2
                for hpair in range(2):
                    nkt = 4 * qc + 4
                    accs = []
                    for half in range(2):
                        acc_t = accps.tile([HD + 1, ROWS], F32, tag="acc")
                        accs.append(acc_t)
                    # lead the exp stream by 2 st_exps so the first PV
                    # pair finds PT2 ready sooner after the phase boundary
                    lead = min(3, 4 * qc)
                    for kt in range(lead):
                        st_exp(qc, hpair, kt)
                    for kt in range(nkt):
                        # this phase's diagonal slots (kt >= 4qc) were
                        # prefetched by the previous chunk (or phase A)
                        if kt + lead < 4 * qc:
                            st_exp(qc, hpair, kt + lead)
                        # PV accumulation, V-stationary: acc[d,q] += V^T P^T.
                        # Diagonal slots only stream their written columns;
                        # the masked-out prefix is exactly zero so skipping
                        # it is exact (and lets PT2 skip zero-memsets).
                        qo = max(0, 128 * kt - 512 * qc)
                        for half in range(2):
                            hl = 2 * hpair + half
                            nc.tensor.matmul(
                                accs[half][:, qo:ROWS],
                                lhsT=V_nat[:, hl, kt, :],
                                rhs=PT2[hpair][:, half, kt, qo:ROWS],
                                start=(kt == 0),
                                stop=(kt == nkt - 1),
                            )
                        # previous chunk's Wp as PE filler, one q-tile per
                        # slot. Slots start at kt=3: earlier fillers stall
                        # the PE queue head on attT2 pair-1 (the previous
                        # chunk's hpair-1 normalize lands ~3us into this
                        # phase). At qc=3 ALL four tiles are held back for
                        # the tail (qc3's loop is already ACT-saturated).
                        # Wp fillers run TWO chunks behind: each boundary's
                        # normalize chain takes ~9.5us to deliver attT2, so
                        # chunk qc-1's tiles would still stall mid-phase —
                        # chunk qc-2's are guaranteed ready. Wp(0,*) in
                        # qc2/hp0, Wp(1,*) in qc2/hp1; Wp(2,*)+Wp(3,*) in
                        # the tail (qc3's loop is already ACT-saturated).
                        if qc == 2 and kt >= 5 and (kt - 5) % 2 == 0:
                            sidx = (kt - 5) // 2
                            if sidx < 4:
                                emit_wp_s(hpair, sidx)
                    # Stage accs to SBUF immediately: the copies are the only
                    # thing the next phase's PV (acc-buffer WAR) waits on, so
                    # they go ahead of the prefetch block in the DVE queue.
                    last = qc == 3 and hpair == 1
                    if last:
                        # held-back Wp(2) tiles: PE work that hides the final
                        # normalize chain's DMA/reciprocal latency and keeps
                        # HAM warm through the tail. Emitted BEFORE the acc
                        # copies so their dependency watermark excludes the
                        # final boundary's DVE chain.
                        for s in range(4):
                            emit_wp_s(2, s)
                    acc_sbs, bcrecs = [], []
                    for half in range(2):
                        acc_sb = bc_pool.tile([HD + 1, ROWS], F32, tag="acc_sb")
                        nc.vector.tensor_copy(acc_sb, accs[half])
                        acc_sbs.append(acc_sb)
                    if qc < 3:
                        # prefetch the NEXT chunk's fresh diagonal slots of
                        # this parity: keeps the exp stream dense through the
                        # normalize/Wp phase boundary (slots are write-once
                        # here, first read by PV(qc+1, hpair))
                        for kt in range(4 * qc + 4, 4 * qc + 8):
                            st_exp(qc + 1, hpair, kt)
                    # normalize: attT2 = acc[0:64] * (1/denom) broadcast
                    for half in range(2):
                        acc_sb = acc_sbs[half]
                        # reciprocal of the denominator row: DVE recip cost
                        # scales with free-size per lane, so pack the 512
                        # values across 128 partitions first (tiny DMAs).
                        # The pack/unpack DMAs ride the sync HWDGE ring —
                        # near-idle in the attention phase and ~2us faster
                        # per transfer than the gpsimd SWDGE ring, which was
                        # stretching every boundary's attT2 chain to ~7.5us.
                        if last:
                            # final boundary: DMA-free den chain — PE
                            # transposes do the pack/unpack (the two small
                            # DMAs cost ~2us each in completion latency and
                            # left the PE idle long enough to re-throttle).
                            # PSUM scratch borrows the accps slots just
                            # freed by the acc_sb copies. bf16 rec path
                            # (~0.3% extra on the normalization, budget 2%).
                            denb = sm_pool.tile([1, ROWS], BF16, tag="rrow")
                            nc.vector.tensor_copy(denb, acc_sb[HD : HD + 1, :])
                            # [128, 4, 2] bf16: each transposed column at
                            # a 4-byte PSUM offset (alignment requirement)
                            ps_d = accps.tile(
                                [128, 4, 2], BF16, tag="acc", name="ps_d"
                            )
                            for s in range(4):
                                nc.tensor.transpose(
                                    ps_d[:, s, 0:1],
                                    denb[:, 128 * s : 128 * (s + 1)],
                                    ident[0:1, 0:1],
                                )
                            rec4 = sm_pool.tile([128, 4], F32, tag="rec128")
                            nc.vector.reciprocal(rec4, ps_d[:, :, 0])
                            rec4b = sm_pool.tile([128, 4], BF16, tag="den128")
                            nc.vector.tensor_copy(rec4b, rec4)
                            ps_r = accps.tile(
                                [128, ROWS], BF16, tag="acc", name="ps_r"
                            )
                            for s in range(4):
                                nc.tensor.transpose(
                                    ps_r[0:1, 128 * s : 128 * (s + 1)],
                                    rec4b[:, s : s + 1],
                                    ident,
                                )
                            rrowb = sm_pool.tile([1, ROWS], BF16, tag="rrow")
                            nc.vector.tensor_copy(rrowb, ps_r[0:1, :])
                            bcrec = bc_pool.tile([64, ROWS], BF16, tag="bcrec")
                            nc.gpsimd.partition_broadcast(bcrec, rrowb)
                            bcrecs.append(bcrec)
                            continue
                        den128 = sm_pool.tile([128, 4], F32, tag="den128")
                        nc.sync.dma_start(
                            out=den128,
                            in_=acc_sb[HD : HD + 1, :].rearrange(
                                "o (p e) -> o p e", p=128
                            ),
                        )
                        rec128 = sm_pool.tile([128, 4], F32, tag="rec128")
                        nc.vector.reciprocal(rec128, den128)
                        rrow = sm_pool.tile([1, ROWS], F32, tag="rrow")
                        nc.sync.dma_start(
                            out=rrow.rearrange("o (p e) -> o p e", p=128),
                            in_=rec128,
                        )
                        bcrec = bc_pool.tile([64, ROWS], F32, tag="bcrec")
                        nc.gpsimd.partition_broadcast(bcrec, rrow)
                        if not last:
                            dst = attT2[
                                64 * half : 64 * (half + 1), qcpar, hpair, :, :
                            ].rearrange("d s w -> d (s w)")
                            nc.vector.tensor_mul(dst, acc_sb[0:HD, :], bcrec)
                        else:
                            bcrecs.append(bcrec)
                    if last:
                        # final phase: normalize per s-tile so each Wp q-tile
                        # can start as soon as its slice is ready (shrinks
                        # the serial tail after the last exp)
                        for s in range(4):
                            for half in range(2):
                                dst = attT2[
                                    64 * half : 64 * (half + 1),
                                    qcpar, hpair, s, :,
                                ]
                                nc.vector.tensor_mul(
                                    dst,
                                    acc_sbs[half][0:HD, 128 * s : 128 * (s + 1)],
                                    bcrecs[half][:, 128 * s : 128 * (s + 1)],
                                )
                            emit_wp_s(3, s)

    nc.compile()
    return nc


def get_program():
    global _PROGRAM
    if _PROGRAM is None:
        _PROGRAM = _build_program()
    return _PROGRAM


def _prep_x(x, dt):
    """[512 rows, 1024] -> x^T block layout [p, a, t], contiguous."""
    xt = np.ascontiguousarray(np.asarray(x, np.float32).T)  # [1024, 512]
    return np.ascontiguousarray(
        xt.reshape(8, 128, ROWS).transpose(1, 0, 2)
    ).astype(dt)


def _prep_w(W, dt):
    """[1024, 1024] -> [dblk, p, a, d] with W[128a+p, 128dblk+d]."""
    Wf = np.asarray(W, np.float32).reshape(8, 128, 8, 128)
    return np.ascontiguousarray(Wf.transpose(2, 1, 0, 3)).astype(dt)


def make_in_maps(q, k, v, Wq, Wk, Wv, Wp):
    mask = _perm_mask_np()
    Wq_b = _prep_w(Wq, F16_NP)
    Wk_b = _prep_w(Wk, F16_NP)
    Wv_b = _prep_w(Wv, BF16_NP)
    Wp_f = np.asarray(Wp, np.float32)
    in_maps = []
    for core in range(N_CORES):
        b, g = divmod(core, 4)
        rows = slice(ROWS * g, ROWS * (g + 1))
        in_maps.append(
            {
                "qT": _prep_x(np.asarray(q[b])[rows], F16_NP),
                "kT": _prep_x(np.asarray(k[b])[rows], F16_NP),
                "vT": _prep_x(np.asarray(v[b])[rows], BF16_NP),
                "Wq": Wq_b,
                "Wk": Wk_b,
                "Wv": Wv_b,
                "Wp": np.ascontiguousarray(
                    Wp_f[HPC * HD * g : HPC * HD * (g + 1)]
                ).astype(BF16_NP),
                "mask": mask,
            }
        )
    return in_maps


def kernel(q, k, v, Wq, Wk, Wv, Wp, _trace=False, _trace_kwargs=None):
    nc = get_program()
    in_maps = make_in_maps(q, k, v, Wq, Wk, Wv, Wp)
    res = run_bass_kernel_spmd(
        nc,
        in_maps,
        core_ids=list(range(N_CORES)),
        trace=_trace,
        **(_trace_kwargs or {}),
    )
    # un-permute device layout [qt, ec, w=8c+tl, e] -> [s=128qt+16tl+c, 1024]
    s_in_tile = np.arange(128)
    w_of_s = 8 * (s_in_tile % 16) + s_in_tile // 16
    outs = []
    for c in range(N_CORES):
        o = res.results[c]["out"].astype(np.float32)  # [16, 2, 128, 512]
        o = o[:, :, w_of_s, :].transpose(0, 2, 1, 3).reshape(S, D)
        outs.append(o)
    full = np.stack(
        [
            outs[0] + outs[1] + outs[2] + outs[3],
            outs[4] + outs[5] + outs[6] + outs[7],
        ]
    ).astype(np.float32)
    if _trace:
        kernel._last_result = res
    return full

